# revision 28
# baseline (speedup 1.0000x reference)
"""Trainium2 Bass kernel for nn_Extractor: depth-map unprojection + ray point
generation + trilinear volume/weight sampling.

Sharding: data-parallel over pixels. The 240x320 image is split into 8
row-bands of 30 rows (9600 pixels) per NeuronCore; the volume and weight
grids are replicated (interleaved into one [X*Y*Z, 2] array so one gather
descriptor fetches vol/weight for a z-pair in a single contiguous 16B read).

Per core the device program:
  1. inverts the 3x3 intrinsics (adjugate), broadcasts inv(K), E, t, origin,
     resolution, eye_v and the ray offsets to all 128 partitions via a
     K=1 matmul against a ones vector,
  2. unprojects its 9600 pixels (75 per partition) to world points, forms
     per-pixel ray directions,
  3. for each of 5 chunks of 15 pixels/partition: builds the 9 ray points,
     computes floor/alpha/sign/corner indices/weights/validity, emits the
     clipped int32 indices and 8 corner weights, computes the gather
     addresses as (x*320+y)*5*64 + z encoded with an exact int32 shift/or
     (fp32 alone cannot hold 25-bit addresses), and issues 540 indirect
     DMAs (128 descriptors each, 16B per descriptor: vol[z],w[z],vol[z+1],
     w[z+1]) against the interleaved volume,
  4. combines gathered pairs with the z-slot-select weights and the masked
     x/y weights into fv/fw.

Outputs are DMA'd per-chunk; the host reassembles the 8 pixel shards.
"""

import numpy as np

import concourse.bass as bass
import concourse.mybir as mybir
import concourse.tile as tile
import concourse.bacc as bacc
from concourse.bass_utils import run_bass_kernel_spmd

AF = mybir.ActivationFunctionType
OP = mybir.AluOpType
F32 = mybir.dt.float32
I32 = mybir.dt.int32

XV = YV = ZV = 320
VPAIR = XV * YV * ZV  # volw is viewed as [VPAIR, 2] f32
R = 9
EPS = 1e-12
P = 128

# mrow slot map (broadcast row)
S_INV = 0    # inv(K) row-major, 9
S_E = 9      # E[:3,:3] row-major, 9
S_T = 18     # E[:3,3], 3
S_O = 21     # origin, 3
S_RES = 24   # 1/resolution
S_EYE = 25   # eye_v, 3
S_OFF = 28   # ray offsets -4..4, 9
S_IH1 = 37   # split-hi of inv[a][1], 3
S_IL1 = 40   # split-lo of inv[a][1], 3
S_IH2 = 43   # split-hi of inv[a][2], 3
S_IL2 = 46   # split-lo of inv[a][2], 3
S_RESV = 49  # resolution (raw)
S_RSH = 50   # split-hi of resolution
S_RSL = 51   # split-lo of resolution
NSLOT = 56

CSPLIT = 4097.0  # Veltkamp split constant for fp32 (2^12 + 1)

_CACHE = {}


def build_program(F=75, CHPX=15, mode="oct"):
    """Build the per-core Bass program. F = pixels per partition,
    CHPX = pixels per partition per chunk.

    mode="pairs": volw is [VPAIR, 2] (v,w interleaved); 4 gathers per point
                  (one per xy-corner pair), 16B descriptors.
    mode="oct":   voct is [VPAIR, 8] holding (v,w) for the 2x2 xy-neighborhood
                  of each voxel; 1 gather per point, 64B descriptors covering
                  all 8 corners of both volumes.
    """
    assert F % CHPX == 0
    nchunks = F // CHPX
    CH = CHPX * R          # points per partition per chunk
    NIJ = CH * 4           # gather indices per partition per chunk

    nc = bacc.Bacc("TRN2", target_bir_lowering=False)

    d_depth = nc.dram_tensor("depth_t", [P, F], F32, kind="ExternalInput")
    d_u = nc.dram_tensor("ugrid", [P, F], F32, kind="ExternalInput")
    d_v = nc.dram_tensor("vgrid", [P, F], F32, kind="ExternalInput")
    d_k = nc.dram_tensor("k9", [1, 9], F32, kind="ExternalInput")
    d_e = nc.dram_tensor("e16", [1, 16], F32, kind="ExternalInput")
    d_o = nc.dram_tensor("org", [1, 3], F32, kind="ExternalInput")
    d_r = nc.dram_tensor("res1", [1, 1], F32, kind="ExternalInput")
    d_volw = nc.dram_tensor("volw", [VPAIR, 8 if mode == "oct" else 2], F32,
                            kind="ExternalInput")

    o_coords = nc.dram_tensor("o_coords", [P, 3 * F], F32, kind="ExternalOutput")
    o_pts = nc.dram_tensor("o_pts", [P, 27 * F], F32, kind="ExternalOutput")
    o_inds = nc.dram_tensor("o_inds", [P, 216 * F], I32, kind="ExternalOutput")
    o_w8 = nc.dram_tensor("o_w8", [P, 72 * F], F32, kind="ExternalOutput")
    o_fv = nc.dram_tensor("o_fv", [P, 9 * F], F32, kind="ExternalOutput")
    o_fw = nc.dram_tensor("o_fw", [P, 9 * F], F32, kind="ExternalOutput")

    ve = None  # set below

    with tile.TileContext(nc) as tc:
        with (
            tc.tile_pool(name="sm", bufs=1) as sm,
            tc.tile_pool(name="px", bufs=1) as px,
            tc.tile_pool(name="ck", bufs=2) as ck,
            tc.tile_pool(name="ps", bufs=1, space="PSUM") as psp,
        ):
            ve = nc.vector
            ac = nc.scalar

            # ---------------- prelude: inv(K), broadcast row ----------------
            ks = sm.tile([1, 9], F32)
            es = sm.tile([1, 16], F32)
            osb = sm.tile([1, 3], F32)
            rsb = sm.tile([1, 1], F32)
            nc.sync.dma_start(out=ks[:], in_=d_k[:, :])
            nc.sync.dma_start(out=es[:], in_=d_e[:, :])
            nc.sync.dma_start(out=osb[:], in_=d_o[:, :])
            nc.sync.dma_start(out=rsb[:], in_=d_r[:, :])

            wk = sm.tile([1, 16], F32)   # C cofactors 0-8, det 9, tmp 10-15
            mrow = sm.tile([1, NSLOT], F32)
            nc.gpsimd.memset(mrow[:], 0.0)

            def k_(i):
                return ks[:, i:i + 1]

            def wk_(i):
                return wk[:, i:i + 1]

            def tt1(out, a, b, op):
                ve.tensor_tensor(out=out, in0=a, in1=b, op=op)

            def vsplit(hi, lo, x, t):
                """Veltkamp split: x = hi + lo with hi on 12 mantissa bits."""
                ve.tensor_scalar(out=t, in0=x, scalar1=CSPLIT, scalar2=None,
                                 op0=OP.mult)
                ve.tensor_tensor(out=hi, in0=t, in1=x, op=OP.subtract)
                ve.tensor_tensor(out=hi, in0=t, in1=hi, op=OP.subtract)
                ve.tensor_tensor(out=lo, in0=x, in1=hi, op=OP.subtract)

            def _prod(out, xd, yd):
                # xd/yd: ("t", ap) or ("s", ap [part,1])
                if xd[0] == "t" and yd[0] == "s":
                    ve.tensor_scalar(out=out, in0=xd[1], scalar1=yd[1],
                                     scalar2=None, op0=OP.mult)
                elif xd[0] == "s" and yd[0] == "t":
                    ve.tensor_scalar(out=out, in0=yd[1], scalar1=xd[1],
                                     scalar2=None, op0=OP.mult)
                else:
                    ve.tensor_tensor(out=out, in0=xd[1], in1=yd[1], op=OP.mult)

            def efma(out, A, B, C, T):
                """out = fma(A, B, C) emulated in fp32, matching
                numpy-order: e = (((ah*bh - p) + ah*bl) + al*bh) + al*bl;
                TwoSum(p, C); out = s + (e + err).
                A/B: ("s", a, ah, al) with [part,1] APs, or ("t", ap) tensors
                (split computed here). C: tensor AP. T: dict of temp APs
                {ah,al,bh,bl,p,e,t2,s,bb} (tensor-shaped)."""
                if A[0] == "t":
                    vsplit(T["ah"], T["al"], A[1], T["t2"])
                    a_, ah, al = ("t", A[1]), ("t", T["ah"]), ("t", T["al"])
                else:
                    a_, ah, al = ("s", A[1]), ("s", A[2]), ("s", A[3])
                if B[0] == "t":
                    vsplit(T["bh"], T["bl"], B[1], T["t2"])
                    b_, bh, bl = ("t", B[1]), ("t", T["bh"]), ("t", T["bl"])
                else:
                    b_, bh, bl = ("s", B[1]), ("s", B[2]), ("s", B[3])
                p, e, t2, s_, bb = T["p"], T["e"], T["t2"], T["s"], T["bb"]
                _prod(p, a_, b_)
                _prod(e, ah, bh)
                ve.tensor_tensor(out=e, in0=e, in1=p, op=OP.subtract)
                _prod(t2, ah, bl)
                ve.tensor_tensor(out=e, in0=e, in1=t2, op=OP.add)
                _prod(t2, al, bh)
                ve.tensor_tensor(out=e, in0=e, in1=t2, op=OP.add)
                _prod(t2, al, bl)
                ve.tensor_tensor(out=e, in0=e, in1=t2, op=OP.add)
                ve.tensor_tensor(out=s_, in0=p, in1=C, op=OP.add)
                ve.tensor_tensor(out=bb, in0=s_, in1=p, op=OP.subtract)
                ve.tensor_tensor(out=t2, in0=s_, in1=bb, op=OP.subtract)
                ve.tensor_tensor(out=t2, in0=p, in1=t2, op=OP.subtract)
                ve.tensor_tensor(out=bb, in0=C, in1=bb, op=OP.subtract)
                ve.tensor_tensor(out=t2, in0=t2, in1=bb, op=OP.add)
                ve.tensor_tensor(out=e, in0=e, in1=t2, op=OP.add)
                ve.tensor_tensor(out=out, in0=s_, in1=e, op=OP.add)

            # cofactors C[i][j] -> wk[i*3+j]
            cof = [
                (4, 8, 5, 7), (5, 6, 3, 8), (3, 7, 4, 6),
                (2, 7, 1, 8), (0, 8, 2, 6), (1, 6, 0, 7),
                (1, 5, 2, 4), (2, 3, 0, 5), (0, 4, 1, 3),
            ]
            for idx, (a, b, c, d) in enumerate(cof):
                tt1(wk_(10), k_(a), k_(b), OP.mult)
                tt1(wk_(11), k_(c), k_(d), OP.mult)
                tt1(wk_(idx), wk_(10), wk_(11), OP.subtract)
            # det = k0*C00 + k1*C01 + k2*C02
            tt1(wk_(10), k_(0), wk_(0), OP.mult)
            tt1(wk_(11), k_(1), wk_(1), OP.mult)
            tt1(wk_(12), wk_(10), wk_(11), OP.add)
            tt1(wk_(13), k_(2), wk_(2), OP.mult)
            tt1(wk_(9), wk_(12), wk_(13), OP.add)
            # inv[m][j] = C[j][m] / det  -> mrow[S_INV + m*3 + j]
            ve.reciprocal(out=wk_(14), in_=wk_(9))
            for m in range(3):
                for j in range(3):
                    tt1(mrow[:, S_INV + m * 3 + j:S_INV + m * 3 + j + 1],
                        wk_(j * 3 + m), wk_(14), OP.mult)
            # E[:3,:3] and t
            for i in range(3):
                for j in range(3):
                    ve.tensor_copy(out=mrow[:, S_E + i * 3 + j:S_E + i * 3 + j + 1],
                                   in_=es[:, i * 4 + j:i * 4 + j + 1])
                ve.tensor_copy(out=mrow[:, S_T + i:S_T + i + 1],
                               in_=es[:, i * 4 + 3:i * 4 + 4])
            ve.tensor_copy(out=mrow[:, S_O:S_O + 3], in_=osb[:, :])
            # S_RES slot holds 1/res; S_RESV raw res + its split
            ve.reciprocal(out=mrow[:, S_RES:S_RES + 1], in_=rsb[:, :])
            ve.tensor_copy(out=mrow[:, S_RESV:S_RESV + 1], in_=rsb[:, :])
            wk2 = sm.tile([1, 12], F32)
            vsplit(mrow[:, S_RSH:S_RSH + 1], mrow[:, S_RSL:S_RSL + 1],
                   rsb[:, :], wk_(10))
            # splits of inv[a][1] and inv[a][2] (fma scalar operands)
            for a in range(3):
                vsplit(mrow[:, S_IH1 + a:S_IH1 + a + 1],
                       mrow[:, S_IL1 + a:S_IL1 + a + 1],
                       mrow[:, S_INV + a * 3 + 1:S_INV + a * 3 + 2], wk_(10))
                vsplit(mrow[:, S_IH2 + a:S_IH2 + a + 1],
                       mrow[:, S_IL2 + a:S_IL2 + a + 1],
                       mrow[:, S_INV + a * 3 + 2:S_INV + a * 3 + 3], wk_(10))
            # eye_v = (t - origin) / res via correctly-rounded divide
            T1 = {k: wk2[:, i:i + 1] for i, k in
                  enumerate(["ah", "al", "bh", "bl", "p", "e", "t2", "s", "bb",
                             "x1", "x2", "x3"])}
            for i in range(3):
                num = wk_(10)
                tt1(num, mrow[:, S_T + i:S_T + i + 1],
                    mrow[:, S_O + i:S_O + i + 1], OP.subtract)
                q0 = wk_(11)
                tt1(q0, num, mrow[:, S_RES:S_RES + 1], OP.mult)
                nq0 = wk_(12)
                ve.tensor_scalar(out=nq0, in0=q0, scalar1=-1.0, scalar2=None,
                                 op0=OP.mult)
                resid = wk_(13)
                efma(resid, ("t", nq0), ("t", rsb[:, :]), num, T1)
                tt1(resid, resid, mrow[:, S_RES:S_RES + 1], OP.mult)
                tt1(mrow[:, S_EYE + i:S_EYE + i + 1], q0, resid, OP.add)
            for i in range(R):
                nc.gpsimd.memset(mrow[:, S_OFF + i:S_OFF + i + 1], float(i - 4))

            ones = sm.tile([1, P], F32)
            nc.gpsimd.memset(ones[:], 1.0)
            psb = psp.tile([P, NSLOT], F32)
            nc.tensor.matmul(out=psb[:], lhsT=ones[:], rhs=mrow[:], start=True, stop=True)
            bc = sm.tile([P, NSLOT], F32)
            ve.tensor_copy(out=bc[:], in_=psb[:])

            def sc(i):
                return bc[:, i:i + 1]

            # ---------------- per-pixel phase ([128, F] tiles) ----------------
            dep = px.tile([P, F], F32)
            ut = px.tile([P, F], F32)
            vt = px.tile([P, F], F32)
            nc.sync.dma_start(out=dep[:], in_=d_depth[:, :])
            nc.sync.dma_start(out=ut[:], in_=d_u[:, :])
            nc.sync.dma_start(out=vt[:], in_=d_v[:, :])

            uz = px.tile([P, F], F32)
            vz = px.tile([P, F], F32)
            ve.tensor_tensor(out=uz[:], in0=ut[:], in1=dep[:], op=OP.mult)
            ve.tensor_tensor(out=vz[:], in0=vt[:], in1=dep[:], op=OP.mult)

            pc = [px.tile([P, F], F32, tag=f"pc{a}", name=f"pc{a}") for a in range(3)]
            t1 = px.tile([P, F], F32, tag="pxt1")
            t2 = px.tile([P, F], F32, tag="pxt2")
            TF = {k: px.tile([P, F], F32, tag=f"tf_{k}", name=f"tf_{k}")[:]
                  for k in ["ah", "al", "bh", "bl", "p", "e", "t2", "s", "bb"]}
            # pts_c matches the reference's FMA accumulation:
            # pc_a = fma(inv[a,2], z, fma(inv[a,1], v*z, fl(inv[a,0]*u*z)))
            for a in range(3):
                ve.tensor_scalar(out=t1[:], in0=uz[:], scalar1=sc(S_INV + a * 3 + 0),
                                 scalar2=None, op0=OP.mult)
                efma(t2[:],
                     ("s", sc(S_INV + a * 3 + 1), sc(S_IH1 + a), sc(S_IL1 + a)),
                     ("t", vz[:]), t1[:], TF)
                efma(pc[a][:],
                     ("s", sc(S_INV + a * 3 + 2), sc(S_IH2 + a), sc(S_IL2 + a)),
                     ("t", dep[:]), t2[:], TF)

            pw = [px.tile([P, F], F32, tag=f"pw{a}", name=f"pw{a}") for a in range(3)]
            for a in range(3):
                ve.tensor_scalar(out=pw[a][:], in0=pc[0][:], scalar1=sc(S_E + a * 3 + 0),
                                 scalar2=None, op0=OP.mult)
                ve.tensor_scalar(out=t1[:], in0=pc[1][:], scalar1=sc(S_E + a * 3 + 1),
                                 scalar2=None, op0=OP.mult)
                ve.tensor_tensor(out=pw[a][:], in0=pw[a][:], in1=t1[:], op=OP.add)
                ve.tensor_scalar(out=t2[:], in0=pc[2][:], scalar1=sc(S_E + a * 3 + 2),
                                 scalar2=None, op0=OP.mult)
                ve.tensor_tensor(out=pw[a][:], in0=pw[a][:], in1=t2[:], op=OP.add)
                ve.tensor_scalar(out=pw[a][:], in0=pw[a][:], scalar1=sc(S_T + a),
                                 scalar2=None, op0=OP.add)

            # coords out, interleaved [x y z] per pixel
            co = px.tile([P, 3 * F], F32)
            co3 = co[:].rearrange("p (f a) -> p f a", a=3)
            for a in range(3):
                ve.tensor_copy(out=co3[:, :, a:a + 1], in_=pw[a][:, :, None])
            nc.sync.dma_start(out=o_coords[:, :], in_=co[:])

            cv = [px.tile([P, F], F32, tag=f"cv{a}", name=f"cv{a}") for a in range(3)]
            dh = [px.tile([P, F], F32, tag=f"dh{a}", name=f"dh{a}") for a in range(3)]
            q0t = px.tile([P, F], F32, tag="q0t")
            nq0t = px.tile([P, F], F32, tag="nq0t")
            # cv = (pw - origin) / res, correctly rounded:
            # q0 = x*(1/res); cv = q0 + (1/res)*fma(-q0, res, x)
            for a in range(3):
                ve.tensor_scalar(out=t1[:], in0=pw[a][:], scalar1=sc(S_O + a),
                                 scalar2=None, op0=OP.subtract)
                ve.tensor_scalar(out=q0t[:], in0=t1[:], scalar1=sc(S_RES),
                                 scalar2=None, op0=OP.mult)
                ve.tensor_scalar(out=nq0t[:], in0=q0t[:], scalar1=-1.0,
                                 scalar2=None, op0=OP.mult)
                efma(t2[:], ("t", nq0t[:]),
                     ("s", sc(S_RESV), sc(S_RSH), sc(S_RSL)), t1[:], TF)
                ve.tensor_scalar(out=t2[:], in0=t2[:], scalar1=sc(S_RES),
                                 scalar2=None, op0=OP.mult)
                ve.tensor_tensor(out=cv[a][:], in0=q0t[:], in1=t2[:], op=OP.add)
                ve.tensor_scalar(out=dh[a][:], in0=cv[a][:], scalar1=sc(S_EYE + a),
                                 scalar2=None, op0=OP.subtract)
            sq = px.tile([P, F], F32, tag="sq")
            ve.tensor_tensor(out=sq[:], in0=dh[0][:], in1=dh[0][:], op=OP.mult)
            ve.tensor_tensor(out=t1[:], in0=dh[1][:], in1=dh[1][:], op=OP.mult)
            ve.tensor_tensor(out=sq[:], in0=sq[:], in1=t1[:], op=OP.add)
            ve.tensor_tensor(out=t2[:], in0=dh[2][:], in1=dh[2][:], op=OP.mult)
            ve.tensor_tensor(out=sq[:], in0=sq[:], in1=t2[:], op=OP.add)
            # correctly-rounded sqrt: y = act_sqrt(s); y += (s - y*y)/(2y)
            yn = px.tile([P, F], F32, tag="yn")
            ac.activation(out=yn[:], in_=sq[:], func=AF.Sqrt)
            ve.tensor_scalar(out=nq0t[:], in0=yn[:], scalar1=-1.0, scalar2=None,
                             op0=OP.mult)
            efma(t2[:], ("t", nq0t[:]), ("t", yn[:]), sq[:], TF)
            ve.reciprocal(out=t1[:], in_=yn[:])
            ve.tensor_scalar(out=t1[:], in0=t1[:], scalar1=0.5, scalar2=None,
                             op0=OP.mult)
            ve.tensor_tensor(out=t2[:], in0=t2[:], in1=t1[:], op=OP.mult)
            ve.tensor_tensor(out=yn[:], in0=yn[:], in1=t2[:], op=OP.add)
            ve.tensor_scalar(out=yn[:], in0=yn[:], scalar1=EPS, scalar2=None,
                             op0=OP.max)
            # dh = dv / norm, correctly rounded per element
            rnm = px.tile([P, F], F32, tag="rnm")
            ve.reciprocal(out=rnm[:], in_=yn[:])
            for a in range(3):
                ve.tensor_tensor(out=q0t[:], in0=dh[a][:], in1=rnm[:], op=OP.mult)
                ve.tensor_scalar(out=nq0t[:], in0=q0t[:], scalar1=-1.0,
                                 scalar2=None, op0=OP.mult)
                efma(t2[:], ("t", nq0t[:]), ("t", yn[:]), dh[a][:], TF)
                ve.tensor_tensor(out=t2[:], in0=t2[:], in1=rnm[:], op=OP.mult)
                ve.tensor_tensor(out=dh[a][:], in0=q0t[:], in1=t2[:], op=OP.add)

            # ---------------- chunk loop ----------------
            for c in range(nchunks):
                sl = slice(c * CHPX, (c + 1) * CHPX)

                # ray points p_a: [P, CH] with point index = f*9 + r
                p3 = []
                for a in range(3):
                    pa = ck.tile([P, CH], F32, tag=f"p{a}")
                    pa3 = pa[:].rearrange("p (f r) -> p f r", r=R)
                    dh_b = dh[a][:, sl][:, :, None].broadcast_to([P, CHPX, R])
                    off_b = bc[:, S_OFF:S_OFF + R][:, None, :].broadcast_to([P, CHPX, R])
                    cv_b = cv[a][:, sl][:, :, None].broadcast_to([P, CHPX, R])
                    ve.tensor_tensor(out=pa3, in0=off_b, in1=dh_b, op=OP.mult)
                    ve.tensor_tensor(out=pa3, in0=pa3, in1=cv_b, op=OP.add)
                    p3.append(pa)

                # ray_pts out (interleaved x,y,z per point)
                pts = ck.tile([P, CH * 3], F32, tag="pts")
                ptsv = pts[:].rearrange("p (n a) -> p n a", a=3)
                for a in range(3):
                    ve.tensor_copy(out=ptsv[:, :, a:a + 1], in_=p3[a][:, :, None])
                nc.sync.dma_start(out=o_pts[:, c * CH * 3:(c + 1) * CH * 3], in_=pts[:])

                # trilinear per axis
                c0 = []
                c1 = []
                w0 = []
                al = []
                mw0 = []
                mw1 = []
                for a in range(3):
                    m = ck.tile([P, CH], F32, tag=f"m{a}")
                    f_ = ck.tile([P, CH], F32, tag=f"f{a}")
                    alp = ck.tile([P, CH], F32, tag=f"al{a}")
                    nb = ck.tile([P, CH], F32, tag=f"nb{a}")
                    g = ck.tile([P, CH], F32, tag=f"g{a}")
                    ca0 = ck.tile([P, CH], F32, tag=f"c0{a}")
                    ca1 = ck.tile([P, CH], F32, tag=f"c1{a}")
                    va0 = ck.tile([P, CH], F32, tag=f"v0{a}")
                    va1 = ck.tile([P, CH], F32, tag=f"v1{a}")
                    wa0 = ck.tile([P, CH], F32, tag=f"w0{a}")
                    ma0 = ck.tile([P, CH], F32, tag=f"mw0{a}")
                    ma1 = ck.tile([P, CH], F32, tag=f"mw1{a}")
                    pa = p3[a]
                    # floor via the fp32 round-to-int trick (no mod/divide on DVE):
                    # r = (p + 1.5*2^23) - 1.5*2^23 rounds to nearest int (sum stays in
                    # [2^23, 2^24) so fp32 ulp is exactly 1); floor = r - (r > p)
                    ve.tensor_scalar(out=f_[:], in0=pa[:], scalar1=12582912.0,
                                     scalar2=-12582912.0, op0=OP.add, op1=OP.add)
                    ve.tensor_tensor(out=m[:], in0=f_[:], in1=pa[:], op=OP.is_gt)
                    ve.tensor_tensor(out=f_[:], in0=f_[:], in1=m[:], op=OP.subtract)
                    ve.tensor_tensor(out=m[:], in0=pa[:], in1=f_[:], op=OP.subtract)
                    # alpha = |m - 0.5| = max(m-0.5, 0.5-m) ; neighbor = sign(0.5 - m)
                    ve.tensor_scalar(out=alp[:], in0=m[:], scalar1=-0.5, scalar2=None,
                                     op0=OP.add)
                    ve.tensor_scalar(out=nb[:], in0=alp[:], scalar1=-1.0, scalar2=None,
                                     op0=OP.mult)
                    ve.tensor_tensor(out=alp[:], in0=alp[:], in1=nb[:], op=OP.max)
                    ve.tensor_scalar(out=nb[:], in0=m[:], scalar1=0.5, scalar2=None,
                                     op0=OP.is_lt)
                    ve.tensor_scalar(out=wa0[:], in0=m[:], scalar1=0.5, scalar2=None,
                                     op0=OP.is_gt)
                    ve.tensor_tensor(out=nb[:], in0=nb[:], in1=wa0[:], op=OP.subtract)
                    ve.tensor_tensor(out=g[:], in0=f_[:], in1=nb[:], op=OP.add)
                    ve.tensor_scalar(out=ca0[:], in0=f_[:], scalar1=0.0, scalar2=319.0,
                                     op0=OP.max, op1=OP.min)
                    ve.tensor_scalar(out=ca1[:], in0=g[:], scalar1=0.0, scalar2=319.0,
                                     op0=OP.max, op1=OP.min)
                    ve.tensor_tensor(out=va0[:], in0=ca0[:], in1=f_[:], op=OP.is_equal)
                    ve.tensor_tensor(out=va1[:], in0=ca1[:], in1=g[:], op=OP.is_equal)
                    ve.tensor_scalar(out=wa0[:], in0=alp[:], scalar1=-1.0, scalar2=1.0,
                                     op0=OP.mult, op1=OP.add)
                    ve.tensor_tensor(out=ma0[:], in0=va0[:], in1=wa0[:], op=OP.mult)
                    ve.tensor_tensor(out=ma1[:], in0=va1[:], in1=alp[:], op=OP.mult)
                    c0.append(ca0)
                    c1.append(ca1)
                    w0.append(wa0)
                    al.append(alp)
                    mw0.append(ma0)
                    mw1.append(ma1)

                # inds out: [point, corner(8), axis(3)] int32 clipped
                indt = ck.tile([P, CH * 24], I32, tag="indt")
                # x: corner bit i selects c_x[i]; corners i*4 + {0..3}
                vx = indt[:].rearrange("p (n i r a) -> p n i r a", i=2, r=4, a=3)
                for i in range(2):
                    src = (c0[0] if i == 0 else c1[0])
                    ve.tensor_copy(
                        out=vx[:, :, i:i + 1, :, 0:1].squeeze(),
                        in_=src[:, :, None, None, None].broadcast_to([P, CH, 1, 4, 1]).squeeze())
                # y: corner bit j -> corners {j*2, j*2+1, j*2+4, j*2+5}
                vy = indt[:].rearrange("p (n b2 j b0 a) -> p n b2 j b0 a", b2=2, j=2, b0=2, a=3)
                for j in range(2):
                    src = (c0[1] if j == 0 else c1[1])
                    ve.tensor_copy(
                        out=vy[:, :, :, j:j + 1, :, 1:2].squeeze(),
                        in_=src[:, :, None, None, None, None].broadcast_to([P, CH, 2, 1, 2, 1]).squeeze())
                # z: corner bit k -> corners {k, k+2, k+4, k+6}
                vz2 = indt[:].rearrange("p (n m k a) -> p n m k a", m=4, k=2, a=3)
                for k in range(2):
                    src = (c0[2] if k == 0 else c1[2])
                    ve.tensor_copy(
                        out=vz2[:, :, :, k:k + 1, 2:3].squeeze(),
                        in_=src[:, :, None, None, None].broadcast_to([P, CH, 4, 1, 1]).squeeze())
                nc.sync.dma_start(out=o_inds[:, c * CH * 24:(c + 1) * CH * 24], in_=indt[:])

                # w8 out (raw weights, no validity)
                wxy = []
                for i in range(2):
                    for j in range(2):
                        t = ck.tile([P, CH], F32, tag=f"wxy{i}{j}")
                        wi = w0[0] if i == 0 else al[0]
                        wj = w0[1] if j == 0 else al[1]
                        ve.tensor_tensor(out=t[:], in0=wi[:], in1=wj[:], op=OP.mult)
                        wxy.append(t)
                w8t = ck.tile([P, CH * 8], F32, tag="w8t")
                w8v = w8t[:].rearrange("p (n c) -> p n c", c=8)
                for i in range(2):
                    for j in range(2):
                        for k in range(2):
                            cidx = i * 4 + j * 2 + k
                            wk_z = w0[2] if k == 0 else al[2]
                            ve.tensor_tensor(out=w8v[:, :, cidx:cidx + 1],
                                             in0=wxy[i * 2 + j][:, :, None],
                                             in1=wk_z[:, :, None], op=OP.mult)
                nc.sync.dma_start(out=o_w8[:, c * CH * 8:(c + 1) * CH * 8], in_=w8t[:])

                if mode == "oct":
                    # slot bases per axis (x/y: min corner; z clamped to 318)
                    xlo = ck.tile([P, CH], F32, tag="xlo")
                    ylo = ck.tile([P, CH], F32, tag="ylo")
                    zs = ck.tile([P, CH], F32, tag="zs")
                    ve.tensor_tensor(out=xlo[:], in0=c0[0][:], in1=c1[0][:], op=OP.min)
                    ve.tensor_tensor(out=ylo[:], in0=c0[1][:], in1=c1[1][:], op=OP.min)
                    ve.tensor_tensor(out=zs[:], in0=c0[2][:], in1=c1[2][:], op=OP.min)
                    ve.tensor_scalar(out=zs[:], in0=zs[:], scalar1=318.0, scalar2=None,
                                     op0=OP.min)

                    # per-axis slot weights (validity folded in via mw tiles)
                    msel = ck.tile([P, CH], F32, tag="msel")
                    tt = ck.tile([P, CH], F32, tag="ttz")
                    u_ax = []
                    for a, baset in enumerate((xlo, ylo, zs)):
                        u0a = ck.tile([P, CH], F32, tag=f"ua0{a}", name=f"ua0{a}")
                        u1a = ck.tile([P, CH], F32, tag=f"ua1{a}", name=f"ua1{a}")
                        ve.tensor_tensor(out=msel[:], in0=c0[a][:], in1=baset[:],
                                         op=OP.is_equal)
                        ve.tensor_tensor(out=u0a[:], in0=msel[:], in1=mw0[a][:],
                                         op=OP.mult)
                        ve.tensor_tensor(out=msel[:], in0=c1[a][:], in1=baset[:],
                                         op=OP.is_equal)
                        ve.tensor_tensor(out=tt[:], in0=msel[:], in1=mw1[a][:],
                                         op=OP.mult)
                        ve.tensor_tensor(out=u0a[:], in0=u0a[:], in1=tt[:], op=OP.add)
                        ve.tensor_tensor(out=u1a[:], in0=mw0[a][:], in1=mw1[a][:],
                                         op=OP.add)
                        ve.tensor_tensor(out=u1a[:], in0=u1a[:], in1=u0a[:],
                                         op=OP.subtract)
                        u_ax.append((u0a, u1a))

                    # xy slot products
                    uxy = []
                    for a2 in range(2):
                        for b2 in range(2):
                            t = ck.tile([P, CH], F32, tag=f"uxy{a2}{b2}",
                                        name=f"uxy{a2}{b2}")
                            ve.tensor_tensor(out=t[:], in0=u_ax[0][a2][:],
                                             in1=u_ax[1][b2][:], op=OP.mult)
                            uxy.append(t)

                    # address: pidx = (xlo*320 + ylo)*320 + zs, exact via int or
                    q = ck.tile([P, CH], F32, tag="q")
                    zhi = ck.tile([P, CH], F32, tag="zhi")
                    zlo = ck.tile([P, CH], F32, tag="zlo")
                    zlo32 = ck.tile([P, CH], I32, tag="zlo32")
                    ve.tensor_scalar(out=q[:], in0=zs[:], scalar1=1.0 / 64.0,
                                     scalar2=None, op0=OP.mult)
                    ve.tensor_scalar(out=zhi[:], in0=q[:], scalar1=12582912.0,
                                     scalar2=-12582912.0, op0=OP.add, op1=OP.add)
                    ve.tensor_tensor(out=zlo[:], in0=zhi[:], in1=q[:], op=OP.is_gt)
                    ve.tensor_tensor(out=zhi[:], in0=zhi[:], in1=zlo[:], op=OP.subtract)
                    ve.tensor_scalar(out=zlo[:], in0=zhi[:], scalar1=-64.0,
                                     scalar2=None, op0=OP.mult)
                    ve.tensor_tensor(out=zlo[:], in0=zs[:], in1=zlo[:], op=OP.add)
                    ve.tensor_copy(out=zlo32[:], in_=zlo[:])
                    idxt = ck.tile([P, CH], I32, tag="idxt")
                    s1 = ck.tile([P, CH], F32, tag="s1")
                    u_ = ck.tile([P, CH], F32, tag="u_")
                    ui = ck.tile([P, CH], I32, tag="ui")
                    ve.tensor_scalar(out=s1[:], in0=xlo[:], scalar1=320.0,
                                     scalar2=None, op0=OP.mult)
                    ve.tensor_tensor(out=s1[:], in0=s1[:], in1=ylo[:], op=OP.add)
                    ve.tensor_scalar(out=u_[:], in0=s1[:], scalar1=5.0, scalar2=None,
                                     op0=OP.mult)
                    ve.tensor_tensor(out=u_[:], in0=u_[:], in1=zhi[:], op=OP.add)
                    ve.tensor_copy(out=ui[:], in_=u_[:])
                    ve.tensor_scalar(out=ui[:], in0=ui[:], scalar1=6, scalar2=None,
                                     op0=OP.arith_shift_left)
                    ve.tensor_tensor(out=idxt[:], in0=ui[:], in1=zlo32[:],
                                     op=OP.bitwise_or)

                    # one indirect DMA per point: 16 f32 (2 z-slots x 8 q)
                    gd = ck.tile([P, CH * 16], F32, tag="gd")
                    for col in range(CH):
                        nc.gpsimd.indirect_dma_start(
                            out=gd[:, col * 16:(col + 1) * 16],
                            out_offset=None,
                            in_=d_volw[:, :],
                            in_offset=bass.IndirectOffsetOnAxis(
                                ap=idxt[:, col:col + 1], axis=0),
                        )

                    # combine: fv/fw over 8 slot combos
                    gdv = gd[:].rearrange("p (n zc q) -> p n zc q", zc=2, q=8)
                    fv = ck.tile([P, CH], F32, tag="fv")
                    fw = ck.tile([P, CH], F32, tag="fw")
                    tva = ck.tile([P, CH], F32, tag="tva")
                    s8 = ck.tile([P, CH], F32, tag="s8")
                    for c2 in range(2):
                        for a2 in range(2):
                            for b2 in range(2):
                                first = (c2 == 0 and a2 == 0 and b2 == 0)
                                ve.tensor_tensor(out=s8[:], in0=uxy[a2 * 2 + b2][:],
                                                 in1=u_ax[2][c2][:], op=OP.mult)
                                qv = (a2 * 2 + b2) * 2
                                gv = gdv[:, :, c2:c2 + 1, qv:qv + 1]
                                gw = gdv[:, :, c2:c2 + 1, qv + 1:qv + 2]
                                ve.tensor_tensor(out=tva[:, :, None, None], in0=gv,
                                                 in1=s8[:, :, None, None], op=OP.mult)
                                if first:
                                    ve.tensor_copy(out=fv[:], in_=tva[:])
                                else:
                                    ve.tensor_tensor(out=fv[:], in0=fv[:], in1=tva[:],
                                                     op=OP.add)
                                ve.tensor_tensor(out=tva[:, :, None, None], in0=gw,
                                                 in1=s8[:, :, None, None], op=OP.mult)
                                if first:
                                    ve.tensor_copy(out=fw[:], in_=tva[:])
                                else:
                                    ve.tensor_tensor(out=fw[:], in0=fw[:], in1=tva[:],
                                                     op=OP.add)

                    nc.sync.dma_start(out=o_fv[:, c * CH:(c + 1) * CH], in_=fv[:])
                    nc.sync.dma_start(out=o_fw[:, c * CH:(c + 1) * CH], in_=fw[:])
                    continue

                # z pair start zs = clamp(min(cz0, cz1), 0, 318)
                zs = ck.tile([P, CH], F32, tag="zs")
                ve.tensor_tensor(out=zs[:], in0=c0[2][:], in1=c1[2][:], op=OP.min)
                ve.tensor_scalar(out=zs[:], in0=zs[:], scalar1=318.0, scalar2=None,
                                 op0=OP.min)
                # z-slot select weights: slot0 holds z=zs, slot1 z=zs+1;
                # corner z_k contributes its (validity-masked) weight to the
                # slot it landed in after clipping.
                msel = ck.tile([P, CH], F32, tag="msel")
                u0 = ck.tile([P, CH], F32, tag="u0")
                u1 = ck.tile([P, CH], F32, tag="u1")
                tt = ck.tile([P, CH], F32, tag="ttz")
                ve.tensor_tensor(out=msel[:], in0=c0[2][:], in1=zs[:], op=OP.is_equal)
                ve.tensor_tensor(out=u0[:], in0=msel[:], in1=mw0[2][:], op=OP.mult)
                ve.tensor_tensor(out=msel[:], in0=c1[2][:], in1=zs[:], op=OP.is_equal)
                ve.tensor_tensor(out=tt[:], in0=msel[:], in1=mw1[2][:], op=OP.mult)
                ve.tensor_tensor(out=u0[:], in0=u0[:], in1=tt[:], op=OP.add)
                ve.tensor_tensor(out=u1[:], in0=mw0[2][:], in1=mw1[2][:], op=OP.add)
                ve.tensor_tensor(out=u1[:], in0=u1[:], in1=u0[:], op=OP.subtract)

                # addresses: pair index = (cx*320 + cy)*320 + zs
                #          = ((cx*320 + cy)*5 + zhi) << 6 | zlo,  zs = zhi*64+zlo
                q = ck.tile([P, CH], F32, tag="q")
                zhi = ck.tile([P, CH], F32, tag="zhi")
                zlo = ck.tile([P, CH], F32, tag="zlo")
                zlo32 = ck.tile([P, CH], I32, tag="zlo32")
                ve.tensor_scalar(out=q[:], in0=zs[:], scalar1=1.0 / 64.0, scalar2=None,
                                 op0=OP.mult)
                ve.tensor_scalar(out=zhi[:], in0=q[:], scalar1=12582912.0,
                                 scalar2=-12582912.0, op0=OP.add, op1=OP.add)
                ve.tensor_tensor(out=zlo[:], in0=zhi[:], in1=q[:], op=OP.is_gt)
                ve.tensor_tensor(out=zhi[:], in0=zhi[:], in1=zlo[:], op=OP.subtract)
                ve.tensor_scalar(out=zlo[:], in0=zhi[:], scalar1=-64.0, scalar2=None,
                                 op0=OP.mult)
                ve.tensor_tensor(out=zlo[:], in0=zs[:], in1=zlo[:], op=OP.add)
                ve.tensor_copy(out=zlo32[:], in_=zlo[:])

                idxt = ck.tile([P, NIJ], I32, tag="idxt")
                idxv = idxt[:].rearrange("p (n ij) -> p n ij", ij=4)
                s1 = ck.tile([P, CH], F32, tag="s1")
                u_ = ck.tile([P, CH], F32, tag="u_")
                ui = ck.tile([P, CH], I32, tag="ui")
                for i in range(2):
                    for j in range(2):
                        cx = c0[0] if i == 0 else c1[0]
                        cy = c0[1] if j == 0 else c1[1]
                        ve.tensor_scalar(out=s1[:], in0=cx[:], scalar1=320.0,
                                         scalar2=None, op0=OP.mult)
                        ve.tensor_tensor(out=s1[:], in0=s1[:], in1=cy[:], op=OP.add)
                        ve.tensor_scalar(out=u_[:], in0=s1[:], scalar1=5.0,
                                         scalar2=None, op0=OP.mult)
                        ve.tensor_tensor(out=u_[:], in0=u_[:], in1=zhi[:], op=OP.add)
                        ve.tensor_copy(out=ui[:], in_=u_[:])
                        ve.tensor_scalar(out=ui[:], in0=ui[:], scalar1=6, scalar2=None,
                                         op0=OP.arith_shift_left)
                        ve.tensor_tensor(out=idxv[:, :, (i * 2 + j):(i * 2 + j) + 1],
                                         in0=ui[:, :, None], in1=zlo32[:, :, None],
                                         op=OP.bitwise_or)

                # the gathers: one indirect DMA per index column
                gd = ck.tile([P, NIJ * 4], F32, tag="gd")
                for col in range(NIJ):
                    nc.gpsimd.indirect_dma_start(
                        out=gd[:, col * 4:(col + 1) * 4],
                        out_offset=None,
                        in_=d_volw[:, :],
                        in_offset=bass.IndirectOffsetOnAxis(
                            ap=idxt[:, col:col + 1], axis=0),
                    )

                # combine: fv/fw
                gdv = gd[:].rearrange("p (n ij s) -> p n ij s", ij=4, s=4)
                fv = ck.tile([P, CH], F32, tag="fv")
                fw = ck.tile([P, CH], F32, tag="fw")
                tva = ck.tile([P, CH], F32, tag="tva")
                tvb = ck.tile([P, CH], F32, tag="tvb")
                mxy = ck.tile([P, CH], F32, tag="mxy")
                for i in range(2):
                    for j in range(2):
                        ij = i * 2 + j
                        ve.tensor_tensor(out=mxy[:], in0=mw0[0][:] if i == 0 else mw1[0][:],
                                         in1=mw0[1][:] if j == 0 else mw1[1][:], op=OP.mult)
                        gv0 = gdv[:, :, ij:ij + 1, 0:1]
                        gw0 = gdv[:, :, ij:ij + 1, 1:2]
                        gv1 = gdv[:, :, ij:ij + 1, 2:3]
                        gw1 = gdv[:, :, ij:ij + 1, 3:4]
                        u0v = u0[:, :, None, None]
                        u1v = u1[:, :, None, None]
                        # volume
                        ve.tensor_tensor(out=tva[:, :, None, None], in0=gv0, in1=u0v,
                                         op=OP.mult)
                        ve.tensor_tensor(out=tvb[:, :, None, None], in0=gv1, in1=u1v,
                                         op=OP.mult)
                        ve.tensor_tensor(out=tva[:], in0=tva[:], in1=tvb[:], op=OP.add)
                        ve.tensor_tensor(out=tva[:], in0=tva[:], in1=mxy[:], op=OP.mult)
                        if ij == 0:
                            ve.tensor_copy(out=fv[:], in_=tva[:])
                        else:
                            ve.tensor_tensor(out=fv[:], in0=fv[:], in1=tva[:], op=OP.add)
                        # weights volume
                        ve.tensor_tensor(out=tva[:, :, None, None], in0=gw0, in1=u0v,
                                         op=OP.mult)
                        ve.tensor_tensor(out=tvb[:, :, None, None], in0=gw1, in1=u1v,
                                         op=OP.mult)
                        ve.tensor_tensor(out=tva[:], in0=tva[:], in1=tvb[:], op=OP.add)
                        ve.tensor_tensor(out=tva[:], in0=tva[:], in1=mxy[:], op=OP.mult)
                        if ij == 0:
                            ve.tensor_copy(out=fw[:], in_=tva[:])
                        else:
                            ve.tensor_tensor(out=fw[:], in0=fw[:], in1=tva[:], op=OP.add)

                nc.sync.dma_start(out=o_fv[:, c * CH:(c + 1) * CH], in_=fv[:])
                nc.sync.dma_start(out=o_fw[:, c * CH:(c + 1) * CH], in_=fw[:])

    nc.compile()
    return nc


MODE = "oct"


def _get_program(F=75, CHPX=15, mode=MODE):
    key = (F, CHPX, mode)
    if key not in _CACHE:
        _CACHE[key] = build_program(F, CHPX, mode)
    return _CACHE[key]


def build_volw(volume, weights, mode=MODE):
    if mode == "pairs":
        volw = np.empty((VPAIR, 2), dtype=np.float32)
        volw[:, 0] = volume.reshape(-1)
        volw[:, 1] = weights.reshape(-1)
        return volw
    # oct: per voxel, (v,w) of the 2x2 xy-neighborhood; x/y edges zero-padded
    oct_ = np.zeros((XV, YV, ZV, 8), dtype=np.float32)
    oct_[..., 0] = volume
    oct_[..., 1] = weights
    oct_[:, :-1, :, 2] = volume[:, 1:, :]
    oct_[:, :-1, :, 3] = weights[:, 1:, :]
    oct_[:-1, :, :, 4] = volume[1:, :, :]
    oct_[:-1, :, :, 5] = weights[1:, :, :]
    oct_[:-1, :-1, :, 6] = volume[1:, 1:, :]
    oct_[:-1, :-1, :, 7] = weights[1:, 1:, :]
    return oct_.reshape(VPAIR, 8)


def _make_core_inputs(depth_shard, intrinsics, extrinsics, origin, resolution,
                      volw, u_flat, v_flat, F):
    return {
        "depth_t": np.ascontiguousarray(depth_shard.reshape(P, F), dtype=np.float32),
        "ugrid": np.ascontiguousarray(u_flat.reshape(P, F), dtype=np.float32),
        "vgrid": np.ascontiguousarray(v_flat.reshape(P, F), dtype=np.float32),
        "k9": np.ascontiguousarray(intrinsics.reshape(1, 9), dtype=np.float32),
        "e16": np.ascontiguousarray(extrinsics.reshape(1, 16), dtype=np.float32),
        "org": np.ascontiguousarray(origin.reshape(1, 3), dtype=np.float32),
        "res1": np.ascontiguousarray(np.asarray(resolution).reshape(1, 1), dtype=np.float32),
        "volw": volw,
    }


def kernel(depth, extrinsics, intrinsics, volume, weights, origin, resolution,
           _trace=False):
    depth = np.asarray(depth, dtype=np.float32)
    extrinsics = np.asarray(extrinsics, dtype=np.float32)
    intrinsics = np.asarray(intrinsics, dtype=np.float32)
    volume = np.asarray(volume, dtype=np.float32)
    weights = np.asarray(weights, dtype=np.float32)
    origin = np.asarray(origin, dtype=np.float32)
    resolution = np.asarray(resolution, dtype=np.float32)

    b, h, w = depth.shape
    assert (b, h, w) == (1, 240, 320)
    n_cores = 8
    rows = h // n_cores          # 30 rows per core
    npx = rows * w               # 9600
    F = npx // P                 # 75

    volw = build_volw(volume, weights, MODE)

    # pixel coordinate grids (row-major over the full image)
    vv, uu = np.meshgrid(np.arange(h, dtype=np.float32),
                         np.arange(w, dtype=np.float32), indexing="ij")

    nc = _get_program(F=F, CHPX=15, mode=MODE)

    in_maps = []
    for k in range(n_cores):
        rsl = slice(k * rows, (k + 1) * rows)
        in_maps.append(_make_core_inputs(
            depth[0, rsl], intrinsics[0], extrinsics[0], origin, resolution,
            volw, uu[rsl], vv[rsl], F))

    res = run_bass_kernel_spmd(nc, in_maps, core_ids=list(range(n_cores)),
                               trace=_trace)
    if _trace:
        kernel.last_exec_time_ns = res.exec_time_ns
        kernel.last_mean_exec_time_ns = res.mean_exec_time_ns

    fv = np.concatenate([r["o_fv"].reshape(npx, R) for r in res.results], axis=0)[None]
    fw = np.concatenate([r["o_fw"].reshape(npx, R) for r in res.results], axis=0)[None]
    ray_pts = np.concatenate([r["o_pts"].reshape(npx, R, 3) for r in res.results],
                             axis=0)[None]
    inds = np.concatenate([r["o_inds"].reshape(npx, R, 8, 3) for r in res.results],
                          axis=0)[None]
    w8 = np.concatenate([r["o_w8"].reshape(npx, R, 8) for r in res.results],
                        axis=0)[None]
    coords = np.concatenate([r["o_coords"].reshape(npx, 3) for r in res.results],
                            axis=0)[None]
    depth_out = depth.reshape(b, h * w)
    return fv, fw, ray_pts, depth_out, inds, w8, coords
